# revision 7
# baseline (speedup 1.0000x reference)
"""Trainium2 Bass kernel for nn_FFTSelector (topk_masking).

Math: the reference's FFT cross-correlation collapses algebraically.
mean over the ifft axis keeps only the DC bin (sum over n of e^{2pi i nf/F}
is F*delta_f0), and the DC bin of rfft(q) is sum(q).  With
u[b,t] = X[b,t].ravel() @ sum_f(Wq[f]) + sum(bq)   (and v likewise for Wk):

    corr[t1,t2] = (1/(129*B)) * sum_b u[b,t1] * v[b,t2]

then diag-mask, per-row top-6, sort selected indices ascending, and gather
X rows.  The heavy work is reading X once (u,v) and the (B,T,K,N,D) gather.

Sharding (8 cores): core c <-> (b = c//2, h = c%2).  Each core computes
u,v for its 144 rows (b, t in [144h, 144h+144)) from a host-transposed
slice, AllGathers u,v (tiny), redundantly computes corr + top-k + index
sort, then gathers its output shard y[b, 144h:144h+144] from its local
X[b] via indirect DMA.  Host stitches the 8 shards.
"""
import sys
import types

import numpy as np

import concourse.bass as bass
import concourse.mybir as mybir
import concourse.tile as tile
from concourse.bass_utils import run_bass_kernel_spmd

N_CORES = 8
B, T, N, D, FF = 4, 288, 207, 152, 256
ND = N * D                      # 31464
NDPAD = 31744                   # 62 * 512 = 62 * 128 * 4
NTILE = 62
TH = T // 2                     # 144 rows per core
KTOP = 6
F_RFFT = FF // 2 + 1            # 129
SCALE = 1.0 / (F_RFFT * B)      # 1/516
NEG = -1.0e30

# 12-CE sorting network for 6 elements (ascending), verified exhaustively.
SORT_NET = [(0, 5), (1, 3), (2, 4), (1, 2), (3, 4), (0, 3),
            (2, 5), (0, 1), (2, 3), (4, 5), (1, 2), (3, 4)]

TRACE = False          # test.py sets this to capture an NTFF profile
LAST_RESULT = None     # BassKernelResults of the most recent run

_MAX_WAITS = 1


def _split_sync_waits(nc):
    """This container's walrus accepts only one sem-wait command per
    instruction.  Rebuild every basic block, hoisting excess waits onto
    same-engine nops inserted immediately before the offending
    instruction (the engine stalls at the same program point, so the
    semantics are unchanged)."""
    import bass_rust

    bbs = list(nc.main_func.blocks)
    snapshots = [list(bb.instructions) for bb in bbs]
    new_lists = []
    for insts in snapshots:
        new = []
        for ins in insts:
            si = ins.sync_info
            waits = list(si.on_wait) if si is not None and si.on_wait else []
            if len(waits) > _MAX_WAITS:
                for i in range(_MAX_WAITS, len(waits), _MAX_WAITS):
                    nop = nc.engines[ins.engine].nop(nofuse=True)
                    nop.ins.sync_info = bass_rust.SyncInfo(
                        on_wait=waits[i : i + _MAX_WAITS], on_update=[]
                    )
                    new.append(nop.ins)
                si.on_wait = waits[:_MAX_WAITS]
            new.append(ins)
        new_lists.append(new)
    for bb, new in zip(bbs, new_lists):
        bb.instructions = new


def _install_ntff_hook():
    """Synthesize antenv.axon_hooks (absent on this image) from
    trn_agent_boot's ctypes fallback so trace=True can capture NTFFs."""
    if "antenv.axon_hooks" in sys.modules:
        return
    try:
        from trn_agent_boot.trn_boot import _ntff_profile_via_ctypes
    except ImportError:
        return
    hook = _ntff_profile_via_ctypes("/opt/axon/libaxon_pjrt.so")
    mod = types.ModuleType("antenv.axon_hooks")
    mod.get_axon_ntff_profile_hook = lambda: hook
    mod.set_axon_ntff_profile_hook = lambda h: None
    sys.modules["antenv.axon_hooks"] = mod


_program_cache = None


def _build_program():
    f32 = mybir.dt.float32
    i32 = mybir.dt.int32
    u32 = mybir.dt.uint32

    nc = bass.Bass(num_devices=N_CORES)
    xb = nc.dram_tensor("xb", [T, ND], f32, kind="ExternalInput")
    xt = nc.dram_tensor("xt", [NTILE, 128, 576], f32, kind="ExternalInput")
    wtp = nc.dram_tensor("wtp", [128, NTILE * 8], f32, kind="ExternalInput")
    bias = nc.dram_tensor("bias", [2, 1], f32, kind="ExternalInput")
    hsel = nc.dram_tensor("hsel", [128, 1], u32, kind="ExternalInput")
    y = nc.dram_tensor("y", [B * TH * KTOP // B, ND], f32, kind="ExternalOutput")
    sv = nc.dram_tensor("sv", [T, KTOP], f32, kind="ExternalOutput")
    si = nc.dram_tensor("si", [T, KTOP], i32, kind="ExternalOutput")

    with tile.TileContext(nc) as tc:
        with tc.tile_pool(name="dram", bufs=1, space="DRAM") as dpool:
            cc_in = dpool.tile([2, TH], f32)
            cc_out = dpool.tile([2 * N_CORES, TH], f32)
            # padded so the strided [128,7] index loads never run off the end
            si_scr = dpool.tile([304, KTOP], i32)

            # ---- phase 1: u,v for this core's 144 rows --------------------
            with (
                tc.tile_pool(name="w1", bufs=1) as w1,
                tc.tile_pool(name="xtp", bufs=4) as xtp,
                tc.tile_pool(name="ps1", bufs=1, space="PSUM") as ps1,
            ):
                wt_sb = w1.tile([128, NTILE * 8], f32)
                nc.sync.dma_start(wt_sb[:], wtp[:, :])
                bias_sb = w1.tile([2, 1], f32)
                nc.sync.dma_start(bias_sb[:], bias[:, :])

                ps = [
                    ps1.tile([2, TH], f32, tag=f"ps{l}", name=f"ps{l}")
                    for l in range(4)
                ]
                for i in range(NTILE):
                    xt_t = xtp.tile([128, 576], f32, tag="xt")
                    nc.sync.dma_start(xt_t[:], xt[i])
                    for l in range(4):
                        nc.tensor.matmul(
                            ps[l][:],
                            lhsT=wt_sb[:, i * 8 + 2 * l : i * 8 + 2 * l + 2],
                            rhs=xt_t[:, TH * l : TH * (l + 1)],
                            start=(i == 0),
                            stop=(i == NTILE - 1),
                        )
                uv = w1.tile([2, TH], f32)
                nc.vector.tensor_copy(uv[:], ps[0][:])
                nc.vector.tensor_add(uv[:], uv[:], ps[1][:])
                nc.vector.tensor_add(uv[:], uv[:], ps[2][:])
                nc.vector.tensor_add(uv[:], uv[:], ps[3][:])
                nc.vector.tensor_scalar(
                    uv[:], uv[:], bias_sb[:, 0:1], None, op0=mybir.AluOpType.add
                )
                nc.sync.dma_start(cc_in[:], uv[:])

            nc.gpsimd.collective_compute(
                "AllGather",
                mybir.AluOpType.bypass,
                replica_groups=[list(range(N_CORES))],
                ins=[cc_in.opt()],
                outs=[cc_out.opt()],
            )

            # ---- phase 2: corr + top-6 + index sort (replicated) ----------
            with (
                tc.tile_pool(name="p2", bufs=1) as p2,
                tc.tile_pool(name="p2s", bufs=2) as p2s,
                tc.tile_pool(name="ps2", bufs=1, space="PSUM") as ps2,
            ):
                u_sb = p2.tile([B, T], f32)
                v_sb = p2.tile([B, T], f32)
                cc4 = cc_out[:].rearrange("(b h q) i -> b q h i", h=2, q=2)
                nc.sync.dma_start(u_sb[:], cc4[:, 0])
                nc.sync.dma_start(v_sb[:], cc4[:, 1])

                val_pack = p2.tile([96, 24], f32)
                idxf = p2.tile([96, 24], f32)
                for ci in range(3):
                    ps_c = ps2.tile([96, T], f32, tag="corrps")
                    nc.tensor.matmul(
                        ps_c[:],
                        lhsT=u_sb[:, 96 * ci : 96 * (ci + 1)],
                        rhs=v_sb[:],
                        start=True,
                        stop=True,
                    )
                    corr_c = p2s.tile([96, T], f32, tag="corr")
                    nc.vector.tensor_scalar(
                        corr_c[:], ps_c[:], SCALE, None, op0=mybir.AluOpType.mult
                    )
                    # diagonal (col == 96*ci + row) -> NEG
                    nc.gpsimd.affine_select(
                        out=corr_c[:],
                        in_=corr_c[:],
                        compare_op=mybir.AluOpType.not_equal,
                        fill=NEG,
                        base=-96 * ci,
                        channel_multiplier=-1,
                        pattern=[[1, T]],
                    )
                    v8 = p2s.tile([96, 8], f32, tag="v8")
                    i8 = p2s.tile([96, 8], u32, tag="i8")
                    nc.vector.max(v8[:], corr_c[:])
                    nc.vector.max_index(i8[:], v8[:], corr_c[:])
                    nc.vector.tensor_copy(val_pack[:, 8 * ci : 8 * ci + 8], v8[:])
                    nc.vector.tensor_copy(idxf[:, 8 * ci : 8 * ci + 8], i8[:])

                def col(t_, i_):
                    return t_[:].rearrange("p (c k) -> p c k", k=8)[:, :, i_]

                for (a, b_) in SORT_NET:
                    cmp_ = p2s.tile([96, 3], u32, tag="cmp")
                    tmp = p2s.tile([96, 3], f32, tag="tmp")
                    vtmp = p2s.tile([96, 3], f32, tag="vtmp")
                    ia_, ja_ = col(idxf, a), col(idxf, b_)
                    va_, vb_ = col(val_pack, a), col(val_pack, b_)
                    nc.vector.tensor_tensor(
                        out=cmp_[:], in0=ia_, in1=ja_, op=mybir.AluOpType.is_gt
                    )
                    nc.vector.tensor_copy(tmp[:], ia_)
                    nc.vector.copy_predicated(tmp[:], cmp_[:], ja_)
                    nc.vector.copy_predicated(ja_, cmp_[:], ia_)
                    nc.vector.tensor_copy(ia_, tmp[:])
                    nc.vector.tensor_copy(vtmp[:], va_)
                    nc.vector.copy_predicated(vtmp[:], cmp_[:], vb_)
                    nc.vector.copy_predicated(vb_, cmp_[:], va_)
                    nc.vector.tensor_copy(va_, vtmp[:])

                si_i32 = p2.tile([96, 24], i32)
                nc.vector.tensor_copy(si_i32[:], idxf[:])
                for ci in range(3):
                    rows = slice(96 * ci, 96 * (ci + 1))
                    nc.sync.dma_start(sv[rows, :], val_pack[:, 8 * ci : 8 * ci + KTOP])
                    nc.sync.dma_start(si[rows, :], si_i32[:, 8 * ci : 8 * ci + KTOP])
                    nc.sync.dma_start(
                        si_scr[rows, :], si_i32[:, 8 * ci : 8 * ci + KTOP]
                    )

            # ---- phase 3: gather y[r] = xb[idx[r]] ------------------------
            with (
                tc.tile_pool(name="g", bufs=1) as gp,
                tc.tile_pool(name="gi", bufs=1) as gip,
            ):
                hsel_sb = gip.tile([128, 1], u32)
                nc.sync.dma_start(hsel_sb[:], hsel[:, :])
                # per-half index columns: ia[p, g] = si_flat[128 g + p]
                si_flat = si_scr[:].rearrange("t k -> (t k)")
                ia = gip.tile([128, 7], i32)
                ib = gip.tile([128, 7], i32)
                nc.sync.dma_start(
                    ia[:], si_flat[0 : 128 * 7].rearrange("(g p) -> p g", p=128)
                )
                nc.sync.dma_start(
                    ib[:], si_flat[864 : 864 + 128 * 7].rearrange("(g p) -> p g", p=128)
                )
                idxt = gip.tile([128, 7], i32)
                nc.vector.tensor_copy(idxt[:], ia[:])
                nc.vector.copy_predicated(
                    idxt[:], hsel_sb[:].to_broadcast([128, 7]), ib[:]
                )
                HALF = ND // 2  # 15732 elems = 62928 B < 64 KiB ISA field
                for g in range(7):
                    n = 128 if g < 6 else 96
                    xg = gp.tile([128, ND], f32, tag="xg")
                    for ch in range(2):
                        nc.gpsimd.indirect_dma_start(
                            out=xg[:n, ch * HALF : (ch + 1) * HALF],
                            out_offset=None,
                            in_=xb[:, :],
                            in_offset=bass.IndirectOffsetOnAxis(
                                ap=idxt[:n, g : g + 1], axis=0
                            ),
                            element_offset=ch * HALF,
                        )
                    nc.sync.dma_start(y[128 * g : 128 * g + n, :], xg[:n, :])

    _split_sync_waits(nc)
    return nc


def kernel(X, Wq, bq, Wk, bk, K):
    global _program_cache, LAST_RESULT
    assert int(K) == KTOP
    X = np.ascontiguousarray(np.asarray(X, dtype=np.float32))
    Wq = np.asarray(Wq, dtype=np.float32)
    Wk = np.asarray(Wk, dtype=np.float32)
    bq = np.asarray(bq, dtype=np.float32)
    bk = np.asarray(bk, dtype=np.float32)

    if _program_cache is None:
        _program_cache = _build_program()
    nc = _program_cache

    wt = np.zeros((NDPAD, 2), np.float32)
    wt[:ND, 0] = Wq.sum(axis=0)
    wt[:ND, 1] = Wk.sum(axis=0)
    wtp = np.ascontiguousarray(
        wt.reshape(NTILE, 128, 4, 2).transpose(1, 0, 2, 3)
    ).reshape(128, NTILE * 8)
    bias = np.array([[bq.sum()], [bk.sum()]], np.float32)

    in_maps = []
    for c in range(N_CORES):
        b, h = c // 2, c % 2
        xb_c = X[b].reshape(T, ND)
        xt_c = np.zeros((NDPAD, TH), np.float32)
        xt_c[:ND] = X[b, h * TH : (h + 1) * TH].reshape(TH, ND).T
        in_maps.append(
            {
                "xb": xb_c,
                "xt": xt_c.reshape(NTILE, 128, 576),
                "wtp": wtp,
                "bias": bias,
                "hsel": np.full((128, 1), h, np.uint32),
            }
        )

    if TRACE:
        _install_ntff_hook()
    res = run_bass_kernel_spmd(
        nc, in_maps, core_ids=list(range(N_CORES)), trace=TRACE
    )
    LAST_RESULT = res

    sv_out = np.asarray(res.results[0]["sv"])
    si_out = np.asarray(res.results[0]["si"]).astype(np.int32)
    y_full = np.empty((B, T, KTOP, N, D), np.float32)
    for c in range(N_CORES):
        b, h = c // 2, c % 2
        y_full[b, h * TH : (h + 1) * TH] = np.asarray(res.results[c]["y"]).reshape(
            TH, KTOP, N, D
        )
    return sv_out, si_out, y_full


# revision 9
# speedup vs baseline: 1.0571x; 1.0571x over previous
"""Trainium2 Bass kernel for nn_FFTSelector (topk_masking).

Math: the reference's FFT cross-correlation collapses algebraically.
mean over the ifft axis keeps only the DC bin (sum over n of e^{2pi i nf/F}
is F*delta_f0), and the DC bin of rfft(q) is sum(q).  With
u[b,t] = X[b,t].ravel() @ sum_f(Wq[f]) + sum(bq)   (and v likewise for Wk):

    corr[t1,t2] = (1/(129*B)) * sum_b u[b,t1] * v[b,t2]

then diag-mask, per-row top-6, sort selected indices ascending, and gather
X rows.  The heavy work is reading X once (u,v) and the (B,T,K,N,D) gather.

Sharding (8 cores): core c <-> (b = c//2, h = c%2).  Each core computes
u,v for its 144 rows (b, t in [144h, 144h+144)) from a host-transposed
slice, AllGathers u,v (tiny), redundantly computes corr + top-k + index
sort, then gathers its output shard y[b, 144h:144h+144] from its local
X[b] via indirect DMA.  Host stitches the 8 shards.
"""
import sys
import types

import numpy as np

import concourse.bass as bass
import concourse.mybir as mybir
import concourse.tile as tile
from concourse.bass_utils import run_bass_kernel_spmd

N_CORES = 8
B, T, N, D, FF = 4, 288, 207, 152, 256
ND = N * D                      # 31464
NDPAD = 31744                   # 62 * 512 = 62 * 128 * 4
NTILE = 62
TH = T // 2                     # 144 rows per core
KTOP = 6
F_RFFT = FF // 2 + 1            # 129
SCALE = 1.0 / (F_RFFT * B)      # 1/516
NEG = -1.0e30

# 12-CE sorting network for 6 elements (ascending), verified exhaustively.
SORT_NET = [(0, 5), (1, 3), (2, 4), (1, 2), (3, 4), (0, 3),
            (2, 5), (0, 1), (2, 3), (4, 5), (1, 2), (3, 4)]

TRACE = False          # test.py sets this to capture an NTFF profile
LAST_RESULT = None     # BassKernelResults of the most recent run

_MAX_WAITS = 1


def _split_sync_waits(nc):
    """This container's walrus accepts only one sem-wait command per
    instruction.  Rebuild every basic block, hoisting excess waits onto
    same-engine nops inserted immediately before the offending
    instruction (the engine stalls at the same program point, so the
    semantics are unchanged)."""
    import bass_rust

    bbs = list(nc.main_func.blocks)
    snapshots = [list(bb.instructions) for bb in bbs]
    new_lists = []
    for insts in snapshots:
        new = []
        for ins in insts:
            si = ins.sync_info
            waits = list(si.on_wait) if si is not None and si.on_wait else []
            if len(waits) > _MAX_WAITS:
                for i in range(_MAX_WAITS, len(waits), _MAX_WAITS):
                    nop = nc.engines[ins.engine].nop(nofuse=True)
                    nop.ins.sync_info = bass_rust.SyncInfo(
                        on_wait=waits[i : i + _MAX_WAITS], on_update=[]
                    )
                    new.append(nop.ins)
                si.on_wait = waits[:_MAX_WAITS]
            new.append(ins)
        new_lists.append(new)
    for bb, new in zip(bbs, new_lists):
        bb.instructions = new


def _install_ntff_hook():
    """Synthesize antenv.axon_hooks (absent on this image) from
    trn_agent_boot's ctypes fallback so trace=True can capture NTFFs."""
    if "antenv.axon_hooks" in sys.modules:
        return
    try:
        from trn_agent_boot.trn_boot import _ntff_profile_via_ctypes
    except ImportError:
        return
    hook = _ntff_profile_via_ctypes("/opt/axon/libaxon_pjrt.so")
    mod = types.ModuleType("antenv.axon_hooks")
    mod.get_axon_ntff_profile_hook = lambda: hook
    mod.set_axon_ntff_profile_hook = lambda h: None
    sys.modules["antenv.axon_hooks"] = mod


_program_cache = None


def _build_program():
    f32 = mybir.dt.float32
    i32 = mybir.dt.int32
    u32 = mybir.dt.uint32

    nc = bass.Bass(num_devices=N_CORES)
    xb = nc.dram_tensor("xb", [T, ND], f32, kind="ExternalInput")
    xt = nc.dram_tensor("xt", [NTILE, 128, 576], f32, kind="ExternalInput")
    wtp = nc.dram_tensor("wtp", [128, NTILE * 8], f32, kind="ExternalInput")
    bias = nc.dram_tensor("bias", [2, 1], f32, kind="ExternalInput")
    hsel = nc.dram_tensor("hsel", [128, 1], u32, kind="ExternalInput")
    y = nc.dram_tensor("y", [B * TH * KTOP // B, ND], f32, kind="ExternalOutput")
    sv = nc.dram_tensor("sv", [T, KTOP], f32, kind="ExternalOutput")
    si = nc.dram_tensor("si", [T, KTOP], i32, kind="ExternalOutput")

    with tile.TileContext(nc) as tc:
        with tc.tile_pool(name="dram", bufs=1, space="DRAM") as dpool:
            cc_in = dpool.tile([2, TH], f32)
            cc_out = dpool.tile([2 * N_CORES, TH], f32)
            # padded so the strided [128,7] index loads never run off the end
            si_scr = dpool.tile([304, KTOP], i32)

            # ---- phase 1: u,v for this core's 144 rows --------------------
            with (
                tc.tile_pool(name="w1", bufs=1) as w1,
                tc.tile_pool(name="xtp", bufs=4) as xtp,
                tc.tile_pool(name="ps1", bufs=1, space="PSUM") as ps1,
            ):
                wt_sb = w1.tile([128, NTILE * 8], f32)
                nc.sync.dma_start(wt_sb[:], wtp[:, :])
                bias_sb = w1.tile([2, 1], f32)
                nc.sync.dma_start(bias_sb[:], bias[:, :])

                # Two wide matmuls per tile: lhsT [128, 4] = (l, q) pairs for
                # one half, rhs [128, 288] = both l's rows.  Only the two
                # diagonal (l == l') blocks of each [4, 288] psum are used,
                # but this streams each xt element through PE once per half
                # instead of once per l (fp32 PE is 4 passes/col, so this
                # halves PE time).
                psA = ps1.tile([4, 2 * TH], f32)
                psB = ps1.tile([4, 2 * TH], f32)
                for i in range(NTILE):
                    xt_t = xtp.tile([128, 576], f32, tag="xt")
                    nc.sync.dma_start(xt_t[:], xt[i])
                    nc.tensor.matmul(
                        psA[:],
                        lhsT=wt_sb[:, i * 8 : i * 8 + 4],
                        rhs=xt_t[:, 0 : 2 * TH],
                        start=(i == 0),
                        stop=(i == NTILE - 1),
                    )
                    nc.tensor.matmul(
                        psB[:],
                        lhsT=wt_sb[:, i * 8 + 4 : i * 8 + 8],
                        rhs=xt_t[:, 2 * TH : 4 * TH],
                        start=(i == 0),
                        stop=(i == NTILE - 1),
                    )
                # extract the 4 diagonal blocks; the (l=1) blocks sit at
                # partitions 2:4 and DVE can't read partition-shifted
                # operands, so copy PSUM->SBUF then shift via SBUF DMA.
                sA = w1.tile([4, 2 * TH], f32)
                sB = w1.tile([4, 2 * TH], f32)
                nc.vector.tensor_copy(sA[:], psA[:])
                nc.vector.tensor_copy(sB[:], psB[:])
                tA = w1.tile([2, TH], f32)
                tB = w1.tile([2, TH], f32)
                nc.sync.dma_start(tA[:], sA[2:4, TH : 2 * TH])
                nc.sync.dma_start(tB[:], sB[2:4, TH : 2 * TH])
                uv = w1.tile([2, TH], f32)
                nc.vector.tensor_add(uv[:], sA[0:2, 0:TH], tA[:])
                nc.vector.tensor_add(uv[:], uv[:], sB[0:2, 0:TH])
                nc.vector.tensor_add(uv[:], uv[:], tB[:])
                nc.vector.tensor_scalar(
                    uv[:], uv[:], bias_sb[:, 0:1], None, op0=mybir.AluOpType.add
                )
                nc.sync.dma_start(cc_in[:], uv[:])

            nc.gpsimd.collective_compute(
                "AllGather",
                mybir.AluOpType.bypass,
                replica_groups=[list(range(N_CORES))],
                ins=[cc_in.opt()],
                outs=[cc_out.opt()],
            )

            # ---- phase 2: corr + top-6 + index sort (replicated) ----------
            with (
                tc.tile_pool(name="p2", bufs=1) as p2,
                tc.tile_pool(name="p2s", bufs=2) as p2s,
                tc.tile_pool(name="ps2", bufs=1, space="PSUM") as ps2,
            ):
                u_sb = p2.tile([B, T], f32)
                v_sb = p2.tile([B, T], f32)
                cc4 = cc_out[:].rearrange("(b h q) i -> b q h i", h=2, q=2)
                nc.sync.dma_start(u_sb[:], cc4[:, 0])
                nc.sync.dma_start(v_sb[:], cc4[:, 1])

                val_pack = p2.tile([96, 24], f32)
                idxf = p2.tile([96, 24], f32)
                for ci in range(3):
                    ps_c = ps2.tile([96, T], f32, tag="corrps")
                    nc.tensor.matmul(
                        ps_c[:],
                        lhsT=u_sb[:, 96 * ci : 96 * (ci + 1)],
                        rhs=v_sb[:],
                        start=True,
                        stop=True,
                    )
                    corr_c = p2s.tile([96, T], f32, tag="corr")
                    nc.vector.tensor_scalar(
                        corr_c[:], ps_c[:], SCALE, None, op0=mybir.AluOpType.mult
                    )
                    # diagonal (col == 96*ci + row) -> NEG
                    nc.gpsimd.affine_select(
                        out=corr_c[:],
                        in_=corr_c[:],
                        compare_op=mybir.AluOpType.not_equal,
                        fill=NEG,
                        base=-96 * ci,
                        channel_multiplier=-1,
                        pattern=[[1, T]],
                    )
                    v8 = p2s.tile([96, 8], f32, tag="v8")
                    i8 = p2s.tile([96, 8], u32, tag="i8")
                    nc.vector.max(v8[:], corr_c[:])
                    nc.vector.max_index(i8[:], v8[:], corr_c[:])
                    nc.vector.tensor_copy(val_pack[:, 8 * ci : 8 * ci + 8], v8[:])
                    nc.vector.tensor_copy(idxf[:, 8 * ci : 8 * ci + 8], i8[:])

                def col(t_, i_):
                    return t_[:].rearrange("p (c k) -> p c k", k=8)[:, :, i_]

                for (a, b_) in SORT_NET:
                    cmp_ = p2s.tile([96, 3], u32, tag="cmp")
                    tmp = p2s.tile([96, 3], f32, tag="tmp")
                    vtmp = p2s.tile([96, 3], f32, tag="vtmp")
                    ia_, ja_ = col(idxf, a), col(idxf, b_)
                    va_, vb_ = col(val_pack, a), col(val_pack, b_)
                    nc.vector.tensor_tensor(
                        out=cmp_[:], in0=ia_, in1=ja_, op=mybir.AluOpType.is_gt
                    )
                    nc.vector.tensor_copy(tmp[:], ia_)
                    nc.vector.copy_predicated(tmp[:], cmp_[:], ja_)
                    nc.vector.copy_predicated(ja_, cmp_[:], ia_)
                    nc.vector.tensor_copy(ia_, tmp[:])
                    nc.vector.tensor_copy(vtmp[:], va_)
                    nc.vector.copy_predicated(vtmp[:], cmp_[:], vb_)
                    nc.vector.copy_predicated(vb_, cmp_[:], va_)
                    nc.vector.tensor_copy(va_, vtmp[:])

                si_i32 = p2.tile([96, 24], i32)
                nc.vector.tensor_copy(si_i32[:], idxf[:])
                for ci in range(3):
                    rows = slice(96 * ci, 96 * (ci + 1))
                    nc.sync.dma_start(sv[rows, :], val_pack[:, 8 * ci : 8 * ci + KTOP])
                    nc.sync.dma_start(si[rows, :], si_i32[:, 8 * ci : 8 * ci + KTOP])
                    nc.sync.dma_start(
                        si_scr[rows, :], si_i32[:, 8 * ci : 8 * ci + KTOP]
                    )

            # ---- phase 3: gather y[r] = xb[idx[r]] ------------------------
            with (
                tc.tile_pool(name="g", bufs=1) as gp,
                tc.tile_pool(name="gi", bufs=1) as gip,
            ):
                hsel_sb = gip.tile([128, 1], u32)
                nc.sync.dma_start(hsel_sb[:], hsel[:, :])
                # per-half index columns: ia[p, g] = si_flat[128 g + p]
                si_flat = si_scr[:].rearrange("t k -> (t k)")
                ia = gip.tile([128, 7], i32)
                ib = gip.tile([128, 7], i32)
                nc.sync.dma_start(
                    ia[:], si_flat[0 : 128 * 7].rearrange("(g p) -> p g", p=128)
                )
                nc.sync.dma_start(
                    ib[:], si_flat[864 : 864 + 128 * 7].rearrange("(g p) -> p g", p=128)
                )
                idxt = gip.tile([128, 7], i32)
                nc.vector.tensor_copy(idxt[:], ia[:])
                nc.vector.copy_predicated(
                    idxt[:], hsel_sb[:].to_broadcast([128, 7]), ib[:]
                )
                # quarter-row chunks (31464 B < the 64 KiB ISA field) with a
                # 4-deep buffer so SWDGE gather-in overlaps HWDGE write-out
                QC = ND // 4  # 7866
                for g in range(7):
                    n = 128 if g < 6 else 96
                    for ch in range(4):
                        xg = gp.tile([128, QC], f32, tag="xg", bufs=4)
                        nc.gpsimd.indirect_dma_start(
                            out=xg[:n, :],
                            out_offset=None,
                            in_=xb[:, :],
                            in_offset=bass.IndirectOffsetOnAxis(
                                ap=idxt[:n, g : g + 1], axis=0
                            ),
                            element_offset=ch * QC,
                        )
                        nc.sync.dma_start(
                            y[128 * g : 128 * g + n, ch * QC : (ch + 1) * QC],
                            xg[:n, :],
                        )

    _split_sync_waits(nc)
    return nc


def kernel(X, Wq, bq, Wk, bk, K):
    global _program_cache, LAST_RESULT
    assert int(K) == KTOP
    X = np.ascontiguousarray(np.asarray(X, dtype=np.float32))
    Wq = np.asarray(Wq, dtype=np.float32)
    Wk = np.asarray(Wk, dtype=np.float32)
    bq = np.asarray(bq, dtype=np.float32)
    bk = np.asarray(bk, dtype=np.float32)

    if _program_cache is None:
        _program_cache = _build_program()
    nc = _program_cache

    wt = np.zeros((NDPAD, 2), np.float32)
    wt[:ND, 0] = Wq.sum(axis=0)
    wt[:ND, 1] = Wk.sum(axis=0)
    wtp = np.ascontiguousarray(
        wt.reshape(NTILE, 128, 4, 2).transpose(1, 0, 2, 3)
    ).reshape(128, NTILE * 8)
    bias = np.array([[bq.sum()], [bk.sum()]], np.float32)

    in_maps = []
    for c in range(N_CORES):
        b, h = c // 2, c % 2
        xb_c = X[b].reshape(T, ND)
        xt_c = np.zeros((NDPAD, TH), np.float32)
        xt_c[:ND] = X[b, h * TH : (h + 1) * TH].reshape(TH, ND).T
        in_maps.append(
            {
                "xb": xb_c,
                "xt": xt_c.reshape(NTILE, 128, 576),
                "wtp": wtp,
                "bias": bias,
                "hsel": np.full((128, 1), h, np.uint32),
            }
        )

    if TRACE:
        _install_ntff_hook()
    res = run_bass_kernel_spmd(
        nc, in_maps, core_ids=list(range(N_CORES)), trace=TRACE
    )
    LAST_RESULT = res

    sv_out = np.asarray(res.results[0]["sv"])
    si_out = np.asarray(res.results[0]["si"]).astype(np.int32)
    y_full = np.empty((B, T, KTOP, N, D), np.float32)
    for c in range(N_CORES):
        b, h = c // 2, c % 2
        y_full[b, h * TH : (h + 1) * TH] = np.asarray(res.results[c]["y"]).reshape(
            TH, KTOP, N, D
        )
    return sv_out, si_out, y_full
